# revision 2
# baseline (speedup 1.0000x reference)
"""Trainium2 Bass kernel for the isotropic-gaussian differentiable renderer.

Math: for pixel p=(x,y) and gaussian g:
    w[g,p] = op_g * exp(-0.5*((x-ax_g)^2+(y-ay_g)^2)/var_g)
    img[p,c] = (sum_g w[g,p]*col_gc) / (sum_g w[g,p] + n_chunks*EPS)

The isotropic RBF is separable: w = op * exp(sx) * exp(sy) with
sx = s*(x-ax)^2, sy = s*(y-ay)^2 + ln(op), s = -0.5/var.  That turns the
268M-element exp into 2*N*128 exps plus matmuls:

  per 128-gaussian chunk:
    PE (f32r): arg[g, 0:128]=sx(g,x), arg[g,128:256]=sy(g,y) via a K=12
               matmul against fixed rows [u^2,u,1|v^2,v,1] duplicated for a
               hi/lo coefficient split (centered coords; the split keeps the
               catastrophically-cancelling quadratic exact in f32r)
    ACT      : exp(arg) -> fp16 written into fused per-chunk blocks
               [expx(128) | B(128) | colors(384)]; the y half lands as the
               den block B = op*expy directly (ln(op) is in the argument)
    DVE+PL   : 3 tensor_scalar ops (2 on Vector, 1 on Pool) fill the color
               blocks col_c*B from the SAME rounded B, so fp16 weight
               rounding cancels in num/den
    PE (fp16): acc[x, (den|c)*128+y] += block[0:128]^T @ block[128:640]
               (fp32 PSUM accumulate)

The coef/rhs DRAM tensors are declared float32r and DMA'd directly into
f32r tiles (the host pre-rounds to the f32r grid, so no on-chip cast is
needed).  The PE is pre-warmed off a memset tile starting right after the
entry barrier -- before any DMA lands -- to flip the HAM clock gate
(4/8 -> 8/8) as early as possible; the main accumulation then streams at
2.4 GHz instead of 1.2.

Sharding: gaussians split 2048/core across 8 cores; every core accumulates
the full 128x128 image; the partial accumulators leave the chip as fp16
(halves the output DMA; fp16 rounding of the partials is ~5e-4 relative,
far inside the 2e-2 gate).  Host sums the 8 partials, divides num/den and
reshapes to the reference's [4,3,64,64] tile layout.
"""
import numpy as np

import concourse.bacc as bacc
import concourse.tile as tile
from concourse import mybir
from concourse.bass_utils import run_bass_kernel_spmd

# Problem constants (hardcoded per harness contract)
N_GAUSS = 16384
H = 128
W = 128
FX = 128.0
FY = 128.0
CX = 64.0
CY = 64.0
EPS = 1e-8
N_CORES = 8
G_PER_CORE = N_GAUSS // N_CORES      # 2048
CHUNK = 128                          # gaussians per matmul chunk
N_CHUNKS = G_PER_CORE // CHUNK       # 16
ARG_W = 256                          # per-chunk arg width: 128 x | 128 y
OUT_W = 512                          # (c,y) free width of the accumulator

F32 = mybir.dt.float32
F16 = mybir.dt.float16
MM_DT = mybir.dt.float16             # main-accumulation matmul dtype.
# fp16 is safe here because of how A is factored: B = op*expy is rounded
# once and BOTH num and den consume the same rounded B (and the same
# rounded expx), so weight-rounding cancels in num/den; only the color
# weights carry an independent 2^-11 rounding, which averages out.
F32R = mybir.dt.float32r
KARG = 12                            # arg-matmul contraction: 6 coef rows x hi/lo
N_WARM = 12                          # PE warmup matmuls (keep HAM busy until coef lands)
WARM_W = 256                         # warmup matmul streaming width


def build_program():
    """One SPMD Bass program; every core runs it on its gaussian slice."""
    nc = bacc.Bacc("TRN2", target_bir_lowering=False, debug=False,
                   num_devices=N_CORES)
    # [12, 2048] flat, one chunk per 128 columns; pre-rounded to the f32r
    # grid on the host so the DMA'd bits are already valid f32r operands.
    coef = nc.dram_tensor("coef", [KARG, G_PER_CORE], F32R,
                          kind="ExternalInput")
    # the 6 fixed moving rows [u^2,u,1|0] / [0|v^2,v,1], hi/lo duplicated.
    rhsxy = nc.dram_tensor("rhsxy", [KARG, ARG_W], F32R, kind="ExternalInput")
    # [128, 64]: opc[p, chunk*4+c] = colors[chunk*128+p, c] (c=3 unused)
    opc = nc.dram_tensor("opc", [128, N_CHUNKS * 4], F32, kind="ExternalInput")
    # partial accumulator: [x, (den|r|g|b)*128+y], fp16 on the wire
    out = nc.dram_tensor("out", [128, OUT_W], F16, kind="ExternalOutput")

    with tile.TileContext(nc) as tc:
        with tc.tile_pool(name="ins", bufs=1) as ins_pool, \
             tc.tile_pool(name="expp", bufs=1) as exp_pool, \
             tc.tile_pool(name="args", bufs=3, space="PSUM") as arg_pool, \
             tc.tile_pool(name="acc", bufs=1, space="PSUM") as acc_pool, \
             tc.tile_pool(name="warmp", bufs=1, space="PSUM") as warm_pool, \
             tc.tile_pool(name="outp", bufs=1) as out_pool:

            # PE warmup source: memset FIRST on Pool so the warmup matmuls
            # can start right after the entry barrier, before any DMA lands.
            wsrc = ins_pool.tile([128, WARM_W], mybir.dt.bfloat16)
            nc.gpsimd.memset(wsrc, 0.0)

            coef_r = ins_pool.tile([KARG, G_PER_CORE], F32R)
            rhs_r = ins_pool.tile([KARG, ARG_W], F32R)
            opc_t = ins_pool.tile([128, N_CHUNKS * 4], F32)
            # input DMA triggers spread over SP and Pool queues; ACT keeps
            # only its act-table load ahead of the exps.  rhs + coef chunks
            # 0-7 land first (they gate the pipeline head).
            HALF = G_PER_CORE // 2
            nc.sync.dma_start(out=rhs_r, in_=rhsxy[:, :])
            nc.sync.dma_start(out=coef_r[:, 0:HALF], in_=coef[:, 0:HALF])
            nc.gpsimd.dma_start(out=opc_t, in_=opc[:, :])
            nc.gpsimd.dma_start(out=coef_r[:, HALF:], in_=coef[:, HALF:])

            # fused per-chunk block [expx(128) | B(128) | colors(384)]:
            # the exp writes [x|y] at block start (y IS B = op*expy), the
            # DVE/PL tensor_scalars write the color blocks, and the main
            # matmul reads lhsT = block[0:128], rhs = block[128:640].
            BLK = 640
            t3 = exp_pool.tile([128, N_CHUNKS, BLK], MM_DT)
            acc = acc_pool.tile([128, OUT_W], F32)

            # PE warmup off the memset tile: sustained matmul activity flips
            # the HAM clock gate to 8/8 (needs ~3.4us of high duty); sized so
            # the PE stays busy until the first coef slice has landed.
            wdst = warm_pool.tile([128, WARM_W], F32)
            for _ in range(N_WARM):
                nc.tensor.matmul(wdst[:, :], wsrc[:, :CHUNK], wsrc[:, :],
                                 start=True, stop=True)

            # narrow leading groups tighten the pipeline front: chunk 0's
            # exp waits on a 1-chunk arg matmul instead of a 4-chunk batch
            group_plan = [(0, 1), (1, 1), (2, 2), (4, 4), (8, 4), (12, 4)]
            for g0c, width in group_plan:
                args = arg_pool.tile([128, width * ARG_W], F32, tag="args")
                for k in range(width):
                    chunk = g0c + k
                    nc.tensor.matmul(
                        args[:, k * ARG_W:(k + 1) * ARG_W],
                        coef_r[:, chunk * CHUNK:(chunk + 1) * CHUNK],
                        rhs_r[:, :],
                        start=True, stop=True,
                    )
                nc.scalar.activation(
                    out=t3[:, g0c:g0c + width, 0:ARG_W],
                    in_=args[:, :width * ARG_W],
                    func=mybir.ActivationFunctionType.Exp,
                )

            for chunk in range(N_CHUNKS):
                # y half of the exp is B = op*expy (ln(op) in the arg);
                # color blocks multiply the SAME rounded B so num/den
                # rounding cancels.  Accumulator column order: [den|r|g|b].
                # r and g on Vector, b on Pool (otherwise idle here).
                nc.vector.tensor_scalar_mul(
                    out=t3[:, chunk, 256:384],
                    in0=t3[:, chunk, 128:256],
                    scalar1=opc_t[:, chunk * 4:chunk * 4 + 1],
                )
                nc.vector.tensor_scalar_mul(
                    out=t3[:, chunk, 384:512],
                    in0=t3[:, chunk, 128:256],
                    scalar1=opc_t[:, chunk * 4 + 1:chunk * 4 + 2],
                )
                nc.gpsimd.tensor_scalar_mul(
                    out=t3[:, chunk, 512:640],
                    in0=t3[:, chunk, 128:256],
                    scalar1=opc_t[:, chunk * 4 + 2:chunk * 4 + 3],
                )
                nc.tensor.matmul(
                    acc[:, :],
                    t3[:, chunk, 0:128],
                    t3[:, chunk, 128:BLK],
                    start=(chunk == 0), stop=(chunk == N_CHUNKS - 1),
                )

            # PSUM -> SBUF copy (DMA can't read PSUM), downcast to fp16 and
            # split across ACT and DVE so the two halves run in parallel;
            # the two output DMAs trigger from ACT and SP in parallel too.
            out_t = out_pool.tile([128, OUT_W], F16)
            nc.scalar.copy(out=out_t[:, :256], in_=acc[:, :256])
            nc.vector.tensor_copy(out_t[:, 256:], acc[:, 256:])
            nc.scalar.dma_start(out=out[:, :256], in_=out_t[:, :256])
            nc.sync.dma_start(out=out[:, 256:], in_=out_t[:, 256:])

    nc.compile()
    return nc


_PROGRAM = None


def _get_program():
    global _PROGRAM
    if _PROGRAM is None:
        _PROGRAM = build_program()
    return _PROGRAM


def _quat2mat(q):
    q = q / np.linalg.norm(q)
    w, x, y, z = q
    return np.array([
        [1 - 2 * (y * y + z * z), 2 * (x * y - z * w), 2 * (x * z + y * w)],
        [2 * (x * y + z * w), 1 - 2 * (x * x + z * z), 2 * (y * z - x * w)],
        [2 * (x * z - y * w), 2 * (y * z + x * w), 1 - 2 * (x * x + y * y)],
    ])


def kernel(positions, colors, opacities, scales, qvec, tvec, tile_hw,
           chunk_gauss, _trace=False):
    positions = np.asarray(positions, dtype=np.float32)
    colors = np.asarray(colors, dtype=np.float32)
    opacities = np.asarray(opacities, dtype=np.float32)
    scales = np.asarray(scales, dtype=np.float32)
    qvec = np.asarray(qvec, dtype=np.float32)
    tvec = np.asarray(tvec, dtype=np.float32)
    tile_hw = int(tile_hw)
    chunk_gauss = int(chunk_gauss)
    n = positions.shape[0]
    assert n == N_GAUSS, f"expected {N_GAUSS} gaussians, got {n}"

    # ---- O(N) per-gaussian prep in float64 (rounds to the same f32 values
    # the reference computes, to well within the exp's own error budget) ----
    R = _quat2mat(qvec.astype(np.float64))
    cam = positions.astype(np.float64) @ R.T + tvec.astype(np.float64)
    ax = cam[:, 0] / cam[:, 2] * FX + CX          # [N] screen x center
    ay = cam[:, 1] / cam[:, 2] * FY + CY          # [N] screen y center
    var = scales[:, 0].astype(np.float64) ** 2
    s = -0.5 / var                                # [N] negative inv 2*var

    # centered coords keep the quadratic-expansion terms small (|u|<=64)
    dx = ax - CX
    dy = ay - CY

    def f32r_round(x):
        """Round to the f32r grid (low 12 mantissa bits of fp32 cleared)."""
        v32 = np.asarray(x, dtype=np.float32).view(np.uint32)
        return ((v32 + 0x800) & np.uint32(0xFFFFF000)).view(np.float32)

    def hilo(x):
        """Split x into f32r-representable hi+lo with hi+lo ~= x to ~2^-24."""
        hi = f32r_round(x).astype(np.float64)
        lo = f32r_round(np.asarray(x, dtype=np.float64) - hi)
        return hi.astype(np.float32), lo.astype(np.float32)

    # K=12 stationary rows per gaussian (hi/lo pairs), for
    #   arg_x = s*u^2 + (-2 s dx)*u + s*dx^2     (u = x - 64)
    #   arg_y = s*v^2 + (-2 s dy)*v + s*dy^2     (v = y - 64)
    # u^2 <= 4096 is exact in f32r (12-bit significand), so hi-row products
    # are exact in the PE and lo rows mop up the residue: the f32r arg
    # matmul matches fp32 to ~1e-6 despite the quadratic cancellation.
    # +ln(op) on the y-constant row makes exp(arg_y) = op*exp_y directly
    op64 = opacities[:, 0].astype(np.float64)
    rows6 = [s, -2.0 * s * dx, s * dx * dx,
             s, -2.0 * s * dy, s * dy * dy + np.log(op64)]
    coef_rows = []
    for r in rows6:
        hi, lo = hilo(r)
        coef_rows.extend([hi, lo])
    coef_full = np.stack(coef_rows).astype(np.float32)   # [12, N]

    u = np.arange(W, dtype=np.float64) - CX
    v = np.arange(H, dtype=np.float64) - CY
    zeros = np.zeros(128)
    ones = np.ones(128)
    rhs_rows = []
    for base in (u * u, u, ones):
        row = f32r_round(np.concatenate([base, zeros]).astype(np.float32))
        rhs_rows.extend([row, row])   # hi and lo coef rows share the base
    for base in (v * v, v, ones):
        row = f32r_round(np.concatenate([zeros, base]).astype(np.float32))
        rhs_rows.extend([row, row])
    rhsxy = np.stack(rhs_rows)                            # [12, 256]

    # [N, 4] = [r, g, b, unused]: op is folded into the exp's y-argument
    opc_full = np.concatenate(
        [colors.astype(np.float64), np.ones((n, 1))], axis=1
    ).astype(np.float32)

    # ---- shard gaussians across the 8 cores ----
    in_maps = []
    for core in range(N_CORES):
        g0 = core * G_PER_CORE
        g1 = g0 + G_PER_CORE
        opc_c = opc_full[g0:g1].reshape(N_CHUNKS, CHUNK, 4)
        opc_c = np.ascontiguousarray(
            opc_c.transpose(1, 0, 2).reshape(CHUNK, N_CHUNKS * 4))
        in_maps.append({
            "coef": np.ascontiguousarray(coef_full[:, g0:g1]),
            "rhsxy": rhsxy,
            "opc": opc_c,
        })

    nc = _get_program()
    res = run_bass_kernel_spmd(nc, in_maps, list(range(N_CORES)),
                               trace=_trace)

    # ---- host reduction: sum per-core partials, divide, reshape ----
    acc = np.zeros((128, 4, 128), dtype=np.float64)   # [x, (den|r|g|b), y]
    for core in range(N_CORES):
        acc += res.results[core]["out"].astype(np.float64).reshape(128, 4, 128)

    num = acc[:, 1:4, :]                          # [x, c, y]
    n_chunks_ref = n // chunk_gauss
    den = acc[:, 0, :] + n_chunks_ref * EPS       # [x, y]
    img = num / den[:, None, :]                   # [x, c, y]
    img = img.transpose(2, 0, 1).reshape(H * W, 3)  # [p=(y,x), c]

    step = tile_hw * tile_hw
    t = (H * W) // step
    out = img.reshape(t, step, 3).transpose(0, 2, 1).reshape(
        t, 3, tile_hw, tile_hw)
    result = out.astype(np.float32)
    if _trace:
        return result, res
    return result


# revision 6
# speedup vs baseline: 1.6044x; 1.6044x over previous
"""Trainium2 Bass kernel for the isotropic-gaussian differentiable renderer.

Math: for pixel p=(x,y) and gaussian g:
    w[g,p] = op_g * exp(-0.5*((x-ax_g)^2+(y-ay_g)^2)/var_g)
    img[p,c] = (sum_g w[g,p]*col_gc) / (sum_g w[g,p] + n_chunks*EPS)

The isotropic RBF is separable: w = op * exp(sx) * exp(sy) with
sx = s*(x-ax)^2, sy = s*(y-ay)^2 + ln(op), s = -0.5/var.  That turns the
268M-element exp into a few matmuls + narrow exps per 128-gaussian chunk:

    PE (f32r): args via K=14 matmuls against fixed basis rows
               [u^2,u,1 | v^2,v | 1_y | 1_yr] (hi/lo coefficient split keeps
               the catastrophically-cancelling quadratic exact in f32r).
               Three per-chunk arg blocks: x, y (const row carries +ln(op))
               and yr (const row carries +ln(op*r)) -- so the exp directly
               yields expx, B = op*expy and Br = op*r*expy.
    ACT      : ONE exp per group of chunks (contiguous PSUM in, contiguous
               SBUF out) -> fp16
    DVE      : 2 tensor_scalar ops per chunk fill Bg/Bb = {g,b}*B from the
               SAME rounded B
    PE (fp16): acc[x, (den|r|g|b)*128+y] += expx^T @ [B|Br|Bg|Bb]
               (fp32 PSUM accumulate; rhs is a stride-w block AP)

t3 group layout (w chunks/group, 128-col sub-blocks):
    [x_0..x_{w-1} | y_0.. | yr_0.. | yg_0.. | yb_0..]
so every matmul PSUM output lands on 512-byte-aligned 128-col blocks (no
PSUM bank crossing), the group exp is one contiguous activation, and the
acc matmul rhs for chunk k is the affine stride-w block slice {y_k, yr_k,
yg_k, yb_k}.

The coef/rhs DRAM tensors are float32r and DMA'd directly into f32r tiles
(host pre-rounds to the f32r grid; no on-chip cast).  The PE is pre-warmed
off a memset tile right after the entry barrier -- before any DMA lands --
to start flipping the HAM clock gate (4/8 -> 8/8) as early as possible.

Sharding: gaussians split 2048/core across 8 cores; every core accumulates
the full 128x128 image; partials leave the chip as fp16 (halves the output
DMA; ~5e-4 relative on the partials, far inside the 2e-2 gate).  Host sums
the 8 partials, divides num/den and reshapes to [4,3,64,64].
"""
import numpy as np

import concourse.bacc as bacc
import concourse.tile as tile
from concourse import mybir
from concourse.bass_utils import run_bass_kernel_spmd

# Problem constants (hardcoded per harness contract)
N_GAUSS = 16384
H = 128
W = 128
FX = 128.0
FY = 128.0
CX = 64.0
CY = 64.0
EPS = 1e-8
N_CORES = 8
G_PER_CORE = N_GAUSS // N_CORES      # 2048
CHUNK = 128                          # gaussians per matmul chunk
N_CHUNKS = G_PER_CORE // CHUNK       # 16
OUT_W = 512                          # (den|r|g|b) x 128 free width of acc

F32 = mybir.dt.float32
F16 = mybir.dt.float16
MM_DT = mybir.dt.float16
F32R = mybir.dt.float32r
KARG = 14                # x quad/lin/const (6) + y quad/lin (4) + cy (2) + cr (2)
RHS_W = 384              # basis cols: x(128) | y(128) | yr(128)
GROUP_PLAN = [2, 2, 4, 4, 4]         # chunks per arg/exp group
N_WARM = 11                          # PE warmup matmuls until coef lands
WARM_W = 256


def build_program():
    """One SPMD Bass program; every core runs it on its gaussian slice."""
    nc = bacc.Bacc("TRN2", target_bir_lowering=False, debug=False,
                   num_devices=N_CORES)
    # [14, 384+2048]: basis rows first (cols 0:384), then per-gaussian coef
    # columns; one merged tensor so one DMA covers the pipeline head.
    # Pre-rounded to the f32r grid on the host.
    CR_W = RHS_W + G_PER_CORE
    cr = nc.dram_tensor("cr", [KARG, CR_W], F32R, kind="ExternalInput")
    # [128, 32]: opc[p, 2*chunk+{0,1}] = colors[chunk*128+p, {g,b}]
    opc = nc.dram_tensor("opc", [128, N_CHUNKS * 2], F32, kind="ExternalInput")
    # partial accumulator: [x, (den|r|g|b)*128+y], fp16 on the wire
    out = nc.dram_tensor("out", [128, OUT_W], F16, kind="ExternalOutput")

    with tile.TileContext(nc) as tc:
        with tc.tile_pool(name="ins", bufs=1) as ins_pool, \
             tc.tile_pool(name="expp", bufs=1) as exp_pool, \
             tc.tile_pool(name="args", bufs=2, space="PSUM") as arg_pool, \
             tc.tile_pool(name="acc", bufs=1, space="PSUM") as acc_pool, \
             tc.tile_pool(name="warmp", bufs=1, space="PSUM") as warm_pool, \
             tc.tile_pool(name="outp", bufs=1) as out_pool:

            # PE warmup source: memset FIRST on Pool so the warmup matmuls
            # start right after the entry barrier, before any DMA lands.
            wsrc = ins_pool.tile([128, WARM_W], mybir.dt.bfloat16)
            nc.gpsimd.memset(wsrc, 0.0)

            cr_t = ins_pool.tile([KARG, CR_W], F32R)
            opc_t = ins_pool.tile([128, N_CHUNKS * 2], F32)
            # head first: basis + chunks 0-3 gate the whole pipeline.
            C0 = RHS_W + 4 * CHUNK                # basis + chunks 0-3
            C1 = RHS_W + 8 * CHUNK                # .. chunks 4-7
            nc.sync.dma_start(out=cr_t[:, 0:C0], in_=cr[:, 0:C0])
            nc.sync.dma_start(out=cr_t[:, C0:C1], in_=cr[:, C0:C1])
            nc.gpsimd.dma_start(out=opc_t, in_=opc[:, :])
            nc.gpsimd.dma_start(out=cr_t[:, C1:], in_=cr[:, C1:])

            # t3: per-group blocks [x_k.. | y_k.. | yr_k.. | yg_k.. | yb_k..]
            # (5*w sub-blocks of 128 fp16 cols per group of w chunks).
            t3 = exp_pool.tile([128, 5 * N_CHUNKS, CHUNK], MM_DT)
            acc = acc_pool.tile([128, OUT_W], F32)

            # PE warmup off the memset tile: sustained matmul activity flips
            # the HAM clock gate to 8/8; sized to keep the PE busy until the
            # first coef slice has landed.
            wdst = warm_pool.tile([128, WARM_W], F32)
            for _ in range(N_WARM):
                nc.tensor.matmul(wdst[:, :], wsrc[:, :CHUNK], wsrc[:, :],
                                 start=True, stop=True)

            rhs_all = cr_t[:, 0:RHS_W]
            coef0 = RHS_W

            g0c = 0
            group_of = []
            for gi, wdt in enumerate(GROUP_PLAN):
                # args layout: [x_0..x_{w-1} | y_0.. | yr_0..] 128-col blocks
                args = arg_pool.tile([128, 3 * wdt, CHUNK], F32, tag="args")
                for k in range(wdt):
                    chunk = g0c + k
                    group_of.append((gi, g0c, wdt, k))
                    c0 = coef0 + chunk * CHUNK
                    # x / y / yr blocks: three single-block matmuls (strided
                    # multi-block PSUM outs are silently mislowered, so each
                    # matmul writes one contiguous 128-col block).
                    for blk in range(3):
                        nc.tensor.matmul(
                            args[:, blk * wdt + k, :],
                            cr_t[:, c0:c0 + CHUNK],
                            rhs_all[:, blk * CHUNK:(blk + 1) * CHUNK],
                            start=True, stop=True,
                        )
                # one contiguous exp per group: [x|y|yr] blocks -> t3 fp16
                t30 = 5 * g0c
                nc.scalar.activation(
                    out=t3[:, t30:t30 + 3 * wdt, :],
                    in_=args[:, :, :],
                    func=mybir.ActivationFunctionType.Exp,
                )
                g0c += wdt

            for chunk in range(N_CHUNKS):
                gi, g0c, wdt, k = group_of[chunk]
                t30 = 5 * g0c
                yb = t30 + wdt + k                 # B block index
                # yg/yb blocks multiply the SAME rounded B so fp16 rounding
                # cancels between num and den.
                nc.vector.tensor_scalar_mul(
                    out=t3[:, t30 + 3 * wdt + k, :],
                    in0=t3[:, yb, :],
                    scalar1=opc_t[:, 2 * chunk:2 * chunk + 1],
                )
                nc.vector.tensor_scalar_mul(
                    out=t3[:, t30 + 4 * wdt + k, :],
                    in0=t3[:, yb, :],
                    scalar1=opc_t[:, 2 * chunk + 1:2 * chunk + 2],
                )
                # acc[x, (den|r|g|b)*128+y] += expx_k^T @ [B|Br|Bg|Bb]_k
                nc.tensor.matmul(
                    acc[:, :],
                    t3[:, t30 + k, :],
                    t3[:, yb:t30 + 5 * wdt:wdt, :],
                    start=(chunk == 0), stop=(chunk == N_CHUNKS - 1),
                )

            # PSUM -> SBUF fp16 copy (DMA can't read PSUM), split ACT/DVE;
            # the two output DMAs trigger from ACT and SP in parallel.
            out_t = out_pool.tile([128, OUT_W], F16)
            nc.scalar.copy(out=out_t[:, :256], in_=acc[:, :256])
            nc.vector.tensor_copy(out_t[:, 256:], acc[:, 256:])
            nc.scalar.dma_start(out=out[:, :256], in_=out_t[:, :256])
            nc.sync.dma_start(out=out[:, 256:], in_=out_t[:, 256:])

    nc.compile()
    return nc


_PROGRAM = None


def _get_program():
    global _PROGRAM
    if _PROGRAM is None:
        _PROGRAM = build_program()
    return _PROGRAM


def _quat2mat(q):
    q = q / np.linalg.norm(q)
    w, x, y, z = q
    return np.array([
        [1 - 2 * (y * y + z * z), 2 * (x * y - z * w), 2 * (x * z + y * w)],
        [2 * (x * y + z * w), 1 - 2 * (x * x + z * z), 2 * (y * z - x * w)],
        [2 * (x * z - y * w), 2 * (y * z + x * w), 1 - 2 * (x * x + y * y)],
    ])


def kernel(positions, colors, opacities, scales, qvec, tvec, tile_hw,
           chunk_gauss, _trace=False):
    positions = np.asarray(positions, dtype=np.float32)
    colors = np.asarray(colors, dtype=np.float32)
    opacities = np.asarray(opacities, dtype=np.float32)
    scales = np.asarray(scales, dtype=np.float32)
    qvec = np.asarray(qvec, dtype=np.float32)
    tvec = np.asarray(tvec, dtype=np.float32)
    tile_hw = int(tile_hw)
    chunk_gauss = int(chunk_gauss)
    n = positions.shape[0]
    assert n == N_GAUSS, f"expected {N_GAUSS} gaussians, got {n}"

    # ---- O(N) per-gaussian prep in float64 (rounds to the same f32 values
    # the reference computes, to well within the exp's own error budget) ----
    R = _quat2mat(qvec.astype(np.float64))
    cam = positions.astype(np.float64) @ R.T + tvec.astype(np.float64)
    ax = cam[:, 0] / cam[:, 2] * FX + CX          # [N] screen x center
    ay = cam[:, 1] / cam[:, 2] * FY + CY          # [N] screen y center
    var = scales[:, 0].astype(np.float64) ** 2
    s = -0.5 / var                                # [N] negative inv 2*var

    # centered coords keep the quadratic-expansion terms small (|u|<=64)
    dx = ax - CX
    dy = ay - CY

    def f32r_round(x):
        """Round to the f32r grid (low 12 mantissa bits of fp32 cleared)."""
        v32 = np.asarray(x, dtype=np.float32).view(np.uint32)
        return ((v32 + 0x800) & np.uint32(0xFFFFF000)).view(np.float32)

    def hilo(x):
        """Split x into f32r-representable hi+lo with hi+lo ~= x to ~2^-24."""
        hi = f32r_round(x).astype(np.float64)
        lo = f32r_round(np.asarray(x, dtype=np.float64) - hi)
        return hi.astype(np.float32), lo.astype(np.float32)

    # K=14 stationary rows per gaussian (hi/lo pairs), for
    #   arg_x  = s*u^2 + (-2 s dx)*u + s*dx^2             (u = x - 64)
    #   arg_y  = s*v^2 + (-2 s dy)*v + s*dy^2 + ln(op)    (v = y - 64)
    #   arg_yr = s*v^2 + (-2 s dy)*v + s*dy^2 + ln(op*r)
    # exp(arg_y) = B = op*expy and exp(arg_yr) = Br = op*r*expy directly.
    op64 = opacities[:, 0].astype(np.float64)
    col64 = colors.astype(np.float64)
    cy = s * dy * dy + np.log(op64)
    cyr = s * dy * dy + np.log(np.maximum(op64 * col64[:, 0], 1e-30))
    rows7 = [s, -2.0 * s * dx, s * dx * dx,
             s, -2.0 * s * dy, cy, cyr]
    coef_rows = []
    for r in rows7:
        hi, lo = hilo(r)
        coef_rows.extend([hi, lo])
    coef_full = np.stack(coef_rows).astype(np.float32)   # [14, N]

    u = np.arange(W, dtype=np.float64) - CX
    v = np.arange(H, dtype=np.float64) - CY
    z = np.zeros(128)
    o = np.ones(128)
    # basis [14, 384]: cols = x(128) | y(128) | yr(128)
    def row3(a, b, c):
        return f32r_round(np.concatenate([a, b, c]).astype(np.float32))
    rhs_rows = []
    for base in (u * u, u, o):            # x quad/lin/const rows (hi+lo)
        r = row3(base, z, z)
        rhs_rows.extend([r, r])
    for base in (v * v, v):               # y quad/lin rows: active on y AND yr
        r = row3(z, base, base)
        rhs_rows.extend([r, r])
    r = row3(z, o, z)                     # cy const rows
    rhs_rows.extend([r, r])
    r = row3(z, z, o)                     # cyr const rows
    rhs_rows.extend([r, r])
    rhs = np.stack(rhs_rows)                              # [14, 384]

    # merged [14, 384+2048] per core: basis | coef columns
    opc_gb = col64[:, 1:3].astype(np.float32)             # [N, 2] = (g, b)

    in_maps = []
    for core in range(N_CORES):
        g0 = core * G_PER_CORE
        g1 = g0 + G_PER_CORE
        cr = np.concatenate([rhs, coef_full[:, g0:g1]], axis=1)
        opc_c = opc_gb[g0:g1].reshape(N_CHUNKS, CHUNK, 2)
        opc_c = np.ascontiguousarray(
            opc_c.transpose(1, 0, 2).reshape(CHUNK, N_CHUNKS * 2))
        in_maps.append({
            "cr": np.ascontiguousarray(cr),
            "opc": opc_c,
        })

    nc = _get_program()
    res = run_bass_kernel_spmd(nc, in_maps, list(range(N_CORES)),
                               trace=_trace)

    # ---- host reduction: sum per-core partials, divide, reshape ----
    acc = np.zeros((128, 4, 128), dtype=np.float64)   # [x, (den|r|g|b), y]
    for core in range(N_CORES):
        acc += res.results[core]["out"].astype(np.float64).reshape(128, 4, 128)

    num = acc[:, 1:4, :]                          # [x, c, y]
    n_chunks_ref = n // chunk_gauss
    den = acc[:, 0, :] + n_chunks_ref * EPS       # [x, y]
    img = num / den[:, None, :]                   # [x, c, y]
    img = img.transpose(2, 0, 1).reshape(H * W, 3)  # [p=(y,x), c]

    step = tile_hw * tile_hw
    t = (H * W) // step
    out = img.reshape(t, step, 3).transpose(0, 2, 1).reshape(
        t, 3, tile_hw, tile_hw)
    result = out.astype(np.float32)
    if _trace:
        return result, res
    return result


# revision 10
# speedup vs baseline: 1.8022x; 1.1233x over previous
"""Trainium2 Bass kernel for the isotropic-gaussian differentiable renderer.

Math: for pixel p=(x,y) and gaussian g:
    w[g,p] = op_g * exp(-0.5*((x-ax_g)^2+(y-ay_g)^2)/var_g)
    img[p,c] = (sum_g w[g,p]*col_gc) / (sum_g w[g,p] + n_chunks*EPS)

The isotropic RBF is separable: w = op * exp(sx) * exp(sy) with
sx = s*(x-ax)^2, sy = s*(y-ay)^2 + ln(op), s = -0.5/var.  That turns the
268M-element exp into a few matmuls + narrow exps per 128-gaussian chunk:

    PE (f32r): args via K=14 matmuls against fixed basis rows
               [u^2,u,1 | v^2,v | 1_y | 1_yr] (hi/lo coefficient split keeps
               the catastrophically-cancelling quadratic exact in f32r).
               Three per-chunk arg blocks: x, y (const row carries +ln(op))
               and yr (const row carries +ln(op*r)) -- so the exp directly
               yields expx, B = op*expy and Br = op*r*expy.
    ACT      : ONE exp per group of chunks (contiguous PSUM in, contiguous
               SBUF out) -> fp16
    DVE      : 2 tensor_scalar ops per chunk fill Bg/Bb = {g,b}*B from the
               SAME rounded B
    PE (fp16): acc[x, (den|r|g|b)*128+y] += expx^T @ [B|Br|Bg|Bb]
               (fp32 PSUM accumulate; rhs is a stride-w block AP)

t3 group layout (w chunks/group, 128-col sub-blocks):
    [x_0..x_{w-1} | y_0.. | yr_0.. | yg_0.. | yb_0..]
so every matmul PSUM output lands on 512-byte-aligned 128-col blocks (no
PSUM bank crossing), the group exp is one contiguous activation, and the
acc matmul rhs for chunk k is the affine stride-w block slice {y_k, yr_k,
yg_k, yb_k}.

The coef/rhs DRAM tensors are float32r and DMA'd directly into f32r tiles
(host pre-rounds to the f32r grid; no on-chip cast).  The PE is pre-warmed
off a memset tile right after the entry barrier -- before any DMA lands --
to start flipping the HAM clock gate (4/8 -> 8/8) as early as possible.

Sharding: gaussians split 2048/core across 8 cores; every core accumulates
the full 128x128 image; partials leave the chip as fp16 (halves the output
DMA; ~5e-4 relative on the partials, far inside the 2e-2 gate).  Host sums
the 8 partials, divides num/den and reshapes to [4,3,64,64].
"""
import numpy as np

import concourse.bacc as bacc
import concourse.tile as tile
from concourse import mybir
from concourse.bass_utils import run_bass_kernel_spmd

# Problem constants (hardcoded per harness contract)
N_GAUSS = 16384
H = 128
W = 128
FX = 128.0
FY = 128.0
CX = 64.0
CY = 64.0
EPS = 1e-8
N_CORES = 8
G_PER_CORE = N_GAUSS // N_CORES      # 2048
CHUNK = 128                          # gaussians per matmul chunk
N_CHUNKS = G_PER_CORE // CHUNK       # 16
OUT_W = 512                          # (den|r|g|b) x 128 free width of acc

F32 = mybir.dt.float32
F16 = mybir.dt.float16
MM_DT = mybir.dt.float16
F32R = mybir.dt.float32r
KARG = 14                # x quad/lin/const (6) + y quad/lin (4) + cy (2) + cr (2)
RHS_W = 384              # basis cols: x(128) | y(128) | yr(128)
GROUP_PLAN = [2, 2, 4, 4, 4]         # chunks per arg/exp group
N_WARM = 9                           # PE warmup matmuls until coef lands
WARM_W = 256


def build_program():
    """One SPMD Bass program; every core runs it on its gaussian slice."""
    nc = bacc.Bacc("TRN2", target_bir_lowering=False, debug=False,
                   num_devices=N_CORES)
    # [14, 384+2048]: basis rows first (cols 0:384), then per-gaussian coef
    # columns; one merged tensor so one DMA covers the pipeline head.
    # Pre-rounded to the f32r grid on the host.
    CR_W = RHS_W + G_PER_CORE
    cr = nc.dram_tensor("cr", [KARG, CR_W], F32R, kind="ExternalInput")
    # [128, 32]: opc[p, 2*chunk+{0,1}] = colors[chunk*128+p, {g,b}]
    opc = nc.dram_tensor("opc", [128, N_CHUNKS * 2], F32, kind="ExternalInput")
    # partial accumulator: [x, (den|r|g|b)*128+y], fp16 on the wire
    out = nc.dram_tensor("out", [128, OUT_W], F16, kind="ExternalOutput")

    with tile.TileContext(nc) as tc:
        with tc.tile_pool(name="ins", bufs=1) as ins_pool, \
             tc.tile_pool(name="expp", bufs=1) as exp_pool, \
             tc.tile_pool(name="args", bufs=2, space="PSUM") as arg_pool, \
             tc.tile_pool(name="acc", bufs=1, space="PSUM") as acc_pool, \
             tc.tile_pool(name="warmp", bufs=1, space="PSUM") as warm_pool, \
             tc.tile_pool(name="outp", bufs=1) as out_pool:

            # PE warmup source: memset FIRST on Pool so the warmup matmuls
            # start right after the entry barrier, before any DMA lands.
            wsrc = ins_pool.tile([128, WARM_W], mybir.dt.bfloat16)
            nc.gpsimd.memset(wsrc, 0.0)

            cr_t = ins_pool.tile([KARG, CR_W], F32R)
            opc_t = ins_pool.tile([128, N_CHUNKS * 2], F32)
            # head first: basis + chunks 0-1 gate the whole pipeline.
            C0 = RHS_W + 2 * CHUNK                # basis + chunks 0-1
            C1 = RHS_W + 8 * CHUNK                # .. chunks 2-7
            nc.sync.dma_start(out=cr_t[:, 0:C0], in_=cr[:, 0:C0])
            nc.sync.dma_start(out=cr_t[:, C0:C1], in_=cr[:, C0:C1])
            nc.gpsimd.dma_start(out=opc_t, in_=opc[:, :])
            nc.gpsimd.dma_start(out=cr_t[:, C1:], in_=cr[:, C1:])

            # t3: per-chunk blocks [x | y | yr | yg | yb] (5 x 128 fp16 cols)
            # so the acc matmul rhs [y|yr|yg|yb] is one contiguous 512 cols.
            t3 = exp_pool.tile([128, N_CHUNKS, 5, CHUNK], MM_DT)
            acc = acc_pool.tile([128, OUT_W], F32)

            # PE warmup off the memset tile: sustained matmul activity flips
            # the HAM clock gate to 8/8; sized to keep the PE busy until the
            # first coef slice has landed.
            wdst = warm_pool.tile([128, WARM_W], F32)
            for _ in range(N_WARM):
                nc.tensor.matmul(wdst[:, :], wsrc[:, :CHUNK], wsrc[:, :],
                                 start=True, stop=True)

            rhs_all = cr_t[:, 0:RHS_W]
            coef0 = RHS_W

            def emit_args(g0c, wdt):
                # one K=14 matmul per chunk: 384 contiguous PSUM cols [x|y|yr]
                args = arg_pool.tile([128, wdt, 3 * CHUNK], F32, tag="args")
                for k in range(wdt):
                    c0 = coef0 + (g0c + k) * CHUNK
                    nc.tensor.matmul(
                        args[:, k, :],
                        cr_t[:, c0:c0 + CHUNK],
                        rhs_all[:, :],
                        start=True, stop=True,
                    )
                # one exp per group; strided out into the per-chunk x|y|yr
                # blocks (ACT handles multi-level output APs fine).
                nc.scalar.activation(
                    out=t3[:, g0c:g0c + wdt, 0:3, :],
                    in_=args[:, :, :],
                    func=mybir.ActivationFunctionType.Exp,
                )

            def emit_accs(g0c, wdt):
                for k in range(wdt):
                    chunk = g0c + k
                    # yg/yb multiply the SAME rounded B so fp16 rounding
                    # cancels between num and den.
                    nc.vector.tensor_scalar_mul(
                        out=t3[:, chunk, 3, :],
                        in0=t3[:, chunk, 1, :],
                        scalar1=opc_t[:, 2 * chunk:2 * chunk + 1],
                    )
                    nc.vector.tensor_scalar_mul(
                        out=t3[:, chunk, 4, :],
                        in0=t3[:, chunk, 1, :],
                        scalar1=opc_t[:, 2 * chunk + 1:2 * chunk + 2],
                    )
                    # acc[x, (den|r|g|b)*128+y] += expx^T @ [B|Br|Bg|Bb]
                    nc.tensor.matmul(
                        acc[:, :],
                        t3[:, chunk, 0, :],
                        t3[:, chunk, 1:5, :],
                        start=(chunk == 0), stop=(chunk == N_CHUNKS - 1),
                    )

            # interleave acc batches between arg groups so the PE's in-order
            # queue overlaps the arg and accumulation phases.
            starts = []
            g0c = 0
            for wdt in GROUP_PLAN:
                starts.append((g0c, wdt))
                g0c += wdt
            emit_args(*starts[0])
            emit_args(*starts[1])
            for gi in range(2, len(starts)):
                emit_args(*starts[gi])
                emit_accs(*starts[gi - 2])
            emit_accs(*starts[-2])
            emit_accs(*starts[-1])

            # PSUM -> SBUF fp16 copy (DMA can't read PSUM), split ACT/DVE;
            # the two output DMAs trigger from ACT and SP in parallel.
            out_t = out_pool.tile([128, OUT_W], F16)
            nc.scalar.copy(out=out_t[:, :256], in_=acc[:, :256])
            nc.vector.tensor_copy(out_t[:, 256:], acc[:, 256:])
            nc.scalar.dma_start(out=out[:, :256], in_=out_t[:, :256])
            nc.sync.dma_start(out=out[:, 256:], in_=out_t[:, 256:])

    nc.compile()
    return nc


_PROGRAM = None


def _get_program():
    global _PROGRAM
    if _PROGRAM is None:
        _PROGRAM = build_program()
    return _PROGRAM


def _quat2mat(q):
    q = q / np.linalg.norm(q)
    w, x, y, z = q
    return np.array([
        [1 - 2 * (y * y + z * z), 2 * (x * y - z * w), 2 * (x * z + y * w)],
        [2 * (x * y + z * w), 1 - 2 * (x * x + z * z), 2 * (y * z - x * w)],
        [2 * (x * z - y * w), 2 * (y * z + x * w), 1 - 2 * (x * x + y * y)],
    ])


def kernel(positions, colors, opacities, scales, qvec, tvec, tile_hw,
           chunk_gauss, _trace=False):
    positions = np.asarray(positions, dtype=np.float32)
    colors = np.asarray(colors, dtype=np.float32)
    opacities = np.asarray(opacities, dtype=np.float32)
    scales = np.asarray(scales, dtype=np.float32)
    qvec = np.asarray(qvec, dtype=np.float32)
    tvec = np.asarray(tvec, dtype=np.float32)
    tile_hw = int(tile_hw)
    chunk_gauss = int(chunk_gauss)
    n = positions.shape[0]
    assert n == N_GAUSS, f"expected {N_GAUSS} gaussians, got {n}"

    # ---- O(N) per-gaussian prep in float64 (rounds to the same f32 values
    # the reference computes, to well within the exp's own error budget) ----
    R = _quat2mat(qvec.astype(np.float64))
    cam = positions.astype(np.float64) @ R.T + tvec.astype(np.float64)
    ax = cam[:, 0] / cam[:, 2] * FX + CX          # [N] screen x center
    ay = cam[:, 1] / cam[:, 2] * FY + CY          # [N] screen y center
    var = scales[:, 0].astype(np.float64) ** 2
    s = -0.5 / var                                # [N] negative inv 2*var

    # centered coords keep the quadratic-expansion terms small (|u|<=64)
    dx = ax - CX
    dy = ay - CY

    def f32r_round(x):
        """Round to the f32r grid (low 12 mantissa bits of fp32 cleared)."""
        v32 = np.asarray(x, dtype=np.float32).view(np.uint32)
        return ((v32 + 0x800) & np.uint32(0xFFFFF000)).view(np.float32)

    def hilo(x):
        """Split x into f32r-representable hi+lo with hi+lo ~= x to ~2^-24."""
        hi = f32r_round(x).astype(np.float64)
        lo = f32r_round(np.asarray(x, dtype=np.float64) - hi)
        return hi.astype(np.float32), lo.astype(np.float32)

    # K=14 stationary rows per gaussian (hi/lo pairs), for
    #   arg_x  = s*u^2 + (-2 s dx)*u + s*dx^2             (u = x - 64)
    #   arg_y  = s*v^2 + (-2 s dy)*v + s*dy^2 + ln(op)    (v = y - 64)
    #   arg_yr = s*v^2 + (-2 s dy)*v + s*dy^2 + ln(op*r)
    # exp(arg_y) = B = op*expy and exp(arg_yr) = Br = op*r*expy directly.
    op64 = opacities[:, 0].astype(np.float64)
    col64 = colors.astype(np.float64)
    cy = s * dy * dy + np.log(op64)
    cyr = s * dy * dy + np.log(np.maximum(op64 * col64[:, 0], 1e-30))
    rows7 = [s, -2.0 * s * dx, s * dx * dx,
             s, -2.0 * s * dy, cy, cyr]
    coef_rows = []
    for r in rows7:
        hi, lo = hilo(r)
        coef_rows.extend([hi, lo])
    coef_full = np.stack(coef_rows).astype(np.float32)   # [14, N]

    u = np.arange(W, dtype=np.float64) - CX
    v = np.arange(H, dtype=np.float64) - CY
    z = np.zeros(128)
    o = np.ones(128)
    # basis [14, 384]: cols = x(128) | y(128) | yr(128)
    def row3(a, b, c):
        return f32r_round(np.concatenate([a, b, c]).astype(np.float32))
    rhs_rows = []
    for base in (u * u, u, o):            # x quad/lin/const rows (hi+lo)
        r = row3(base, z, z)
        rhs_rows.extend([r, r])
    for base in (v * v, v):               # y quad/lin rows: active on y AND yr
        r = row3(z, base, base)
        rhs_rows.extend([r, r])
    r = row3(z, o, z)                     # cy const rows
    rhs_rows.extend([r, r])
    r = row3(z, z, o)                     # cyr const rows
    rhs_rows.extend([r, r])
    rhs = np.stack(rhs_rows)                              # [14, 384]

    # merged [14, 384+2048] per core: basis | coef columns
    opc_gb = col64[:, 1:3].astype(np.float32)             # [N, 2] = (g, b)

    in_maps = []
    for core in range(N_CORES):
        g0 = core * G_PER_CORE
        g1 = g0 + G_PER_CORE
        cr = np.concatenate([rhs, coef_full[:, g0:g1]], axis=1)
        opc_c = opc_gb[g0:g1].reshape(N_CHUNKS, CHUNK, 2)
        opc_c = np.ascontiguousarray(
            opc_c.transpose(1, 0, 2).reshape(CHUNK, N_CHUNKS * 2))
        in_maps.append({
            "cr": np.ascontiguousarray(cr),
            "opc": opc_c,
        })

    nc = _get_program()
    res = run_bass_kernel_spmd(nc, in_maps, list(range(N_CORES)),
                               trace=_trace)

    # ---- host reduction: sum per-core partials, divide, reshape ----
    acc = np.zeros((128, 4, 128), dtype=np.float64)   # [x, (den|r|g|b), y]
    for core in range(N_CORES):
        acc += res.results[core]["out"].astype(np.float64).reshape(128, 4, 128)

    num = acc[:, 1:4, :]                          # [x, c, y]
    n_chunks_ref = n // chunk_gauss
    den = acc[:, 0, :] + n_chunks_ref * EPS       # [x, y]
    img = num / den[:, None, :]                   # [x, c, y]
    img = img.transpose(2, 0, 1).reshape(H * W, 3)  # [p=(y,x), c]

    step = tile_hw * tile_hw
    t = (H * W) // step
    out = img.reshape(t, step, 3).transpose(0, 2, 1).reshape(
        t, 3, tile_hw, tile_hw)
    result = out.astype(np.float32)
    if _trace:
        return result, res
    return result
